# revision 23
# baseline (speedup 1.0000x reference)
"""GQA attention (S=2048, D=2048, 32 q-heads / 8 kv-heads, rope, causal) on 8
Trainium2 NeuronCores, tensor-parallel over heads (1 kv head + 4 q heads per
core). No collectives: each core computes a full (2048, 2048) partial output
attn_c.T @ wo_c (f16) and the host sums the 8 partials (unsharding a
head-sharded contraction).

Layout notes (activations on-chip live in the transposed/"T" domain):
 - xT (D,S) host-transposed so the contraction dim D is the SBUF partition dim.
 - q weights are column-permuted per head PAIR: [hA_ev|hB_ev|hA_od|hB_od] so
   rope becomes 2 full-width muls + 4 combines per pair psum; scores are
   permutation-invariant per head. k likewise [ev|od].
 - qT pair tiles hold head A in partitions 0-63, head B in 64-127; kT2 holds
   the kv head duplicated in both halves -> the two heads' K=64 score matmuls
   run concurrently in different PE row-groups (tile_position auto-derived
   from base partitions).
 - kT2/qTp/v128/attnT are split per 512-column chunk so tile-granular
   dependencies don't serialize stages: attention q-tile t only waits on
   projection chunk t, and out s-block sb only waits on attention tile
   t = sb//4 of each head.
 - scoresT[s,q] per 128-row s-block; the pair's two score blocks land in one
   [128,2,512] PSUM tile so a single ACTIVATE exps both (halves ACT count).
 - softmax denominators come free from a ones-row appended to v (row 64 of the
   PV psum); no max-subtraction (scores*0.125 ~ N(0,1), exp safe in f32).
 - causal masking: s-blocks strictly above the diagonal are skipped; the
   diagonal 128x128 sub-block gets a 0/1 mask multiply post-exp.
 - stage W s-blocks are interspersed into late attention tiles to fill PE
   gaps while ACT works through the exps.
"""
import os
import sys
from contextlib import ExitStack

import numpy as np

try:
    import concourse.bass as bass  # noqa: F401
except ImportError:  # platform tree not on sys.path in a fresh dir
    sys.path.insert(0, "/opt/trn_rl_repo")
    import concourse.bass as bass  # noqa: F401

import concourse.mybir as mybir
from concourse import bacc, bass_utils, tile
from concourse.masks import make_identity

F32 = mybir.dt.float32
F16 = mybir.dt.float16
BF16 = mybir.dt.bfloat16
AF = mybir.ActivationFunctionType

S = 2048          # sequence length
D = 2048          # model dim
HD = 64           # head dim
N_CORES = 8
NPAIR = 2         # head pairs per core (4 q heads)


def _build():
    nc = bacc.Bacc("TRN2", target_bir_lowering=False, debug=False,
                   num_devices=N_CORES)
    xT_d = nc.dram_tensor("xT", [4, 16, 128, 512], BF16, kind="ExternalInput")
    wq_d = nc.dram_tensor("wq", [128, 16, NPAIR, 128], BF16,
                          kind="ExternalInput")
    wkv_d = nc.dram_tensor("wkv", [128, 16, 128], BF16, kind="ExternalInput")
    wo_d = nc.dram_tensor("wo", [128, NPAIR, D], BF16, kind="ExternalInput")
    cs_d = nc.dram_tensor("csT", [128, S], F32, kind="ExternalInput")
    sc_d = nc.dram_tensor("scT", [128, S], F32, kind="ExternalInput")
    mask_d = nc.dram_tensor("maskM", [128, 128], BF16, kind="ExternalInput")
    id_d = nc.dram_tensor("idT", [128, 128], BF16, kind="ExternalInput")
    out_d = nc.dram_tensor("out", [16, 128, D], F16, kind="ExternalOutput")

    with tile.TileContext(nc) as tc, ExitStack() as top:
        persist = top.enter_context(tc.tile_pool(name="persist", bufs=1))
        # per-sq-chunk tiles (fine-grained cross-stage deps)
        qTp = [[persist.tile([128, 512], BF16, name=f"qTp{m}_{sq}",
                             uniquify=False) for sq in range(4)]
               for m in range(NPAIR)]
        kT2 = [persist.tile([128, 512], BF16, name=f"kT2_{sq}",
                            uniquify=False) for sq in range(4)]
        v128 = [persist.tile([128, 4, HD + 1], BF16, name=f"v128_{sq}",
                             uniquify=False) for sq in range(4)]
        # attnT[pair][t]: [128, 512]; rows 0-63 head 2p, 64-127 head 2p+1
        attnT = [[persist.tile([128, 512], BF16, name=f"attnT{p}{t}",
                               uniquify=False) for t in range(4)]
                 for p in range(NPAIR)]
        maskM_sb = persist.tile([128, 128], BF16, name="maskM_sb")
        idT_sb = persist.tile([128, 128], BF16, name="idT_sb")
        wo_sb = persist.tile([128, NPAIR, D], BF16, name="wo_sb")
        cs_sb = persist.tile([128, S], F32, name="cs_sb")
        sc_sb = persist.tile([128, S], F32, name="sc_sb")
        ones_sb = persist.tile([128, 512], BF16, name="ones_sb")
        nc.vector.memset(ones_sb[:], 1.0)

        # ---------------- Stage P: q/k/v projections + rope ----------------
        with ExitStack() as ctx:
            wpool = ctx.enter_context(tc.tile_pool(name="wpool", bufs=1))
            wq_sb = wpool.tile([128, 16, NPAIR, 128], BF16, name="wq_sb")
            wkv_sb = wpool.tile([128, 16, 128], BF16, name="wkv_sb")
            vT = wpool.tile([HD + 1, S], F32, name="vT")
            identity = wpool.tile([128, 128], F32, name="identity")
            make_identity(nc, identity[:])
            # per-kc wq chunks so the kc=0 matmul starts after 64KB, not 1MB
            for kc in range(16):
                eng = nc.sync if kc % 2 == 0 else nc.scalar
                eng.dma_start(wq_sb[:, kc], wq_d.ap()[:, kc])
            nc.scalar.dma_start(wkv_sb[:], wkv_d.ap())

            xtb_pool = ctx.enter_context(tc.tile_pool(name="xtb", bufs=10))
            pq_pool = ctx.enter_context(
                tc.tile_pool(name="pq", bufs=4, space="PSUM"))
            pkv_pool = ctx.enter_context(
                tc.tile_pool(name="pkv", bufs=2, space="PSUM"))
            pvt_pool = ctx.enter_context(
                tc.tile_pool(name="pvt", bufs=2, space="PSUM"))
            tmp_pool = ctx.enter_context(tc.tile_pool(name="ropetmp", bufs=2))

            nc.vector.memset(vT[HD:HD + 1, :], 1.0)

            for sq in range(4):
                s0 = 512 * sq
                pq = [pq_pool.tile([128, 512], F32, name=f"pq{sq}_{m}",
                                   tag="pq") for m in range(NPAIR)]
                pkv = pkv_pool.tile([128, 512], F32, name=f"pkv{sq}",
                                    tag="pkv")
                for kc in range(16):
                    xtb = xtb_pool.tile([128, 512], BF16,
                                        name=f"xtb{sq}_{kc}", tag="xtb")
                    eng = nc.sync if kc % 2 == 0 else nc.scalar
                    eng.dma_start(xtb[:], xT_d.ap()[sq, kc])
                    if sq == 0 and kc == 15:
                        # prefetches AFTER the hot-path DMAs so their
                        # descriptors don't grab the SDMA engines first
                        nc.gpsimd.dma_start(maskM_sb[:], mask_d.ap())
                        nc.gpsimd.dma_start(idT_sb[:], id_d.ap())
                        nc.gpsimd.dma_start(cs_sb[:], cs_d.ap())
                        nc.gpsimd.dma_start(sc_sb[:], sc_d.ap())
                        nc.gpsimd.dma_start(wo_sb[:], wo_d.ap())
                    st, sp = (kc == 0), (kc == 15)
                    for m in range(NPAIR):
                        nc.tensor.matmul(pq[m][:], wq_sb[:, kc, m, :],
                                         xtb[:], start=st, stop=sp)
                    nc.tensor.matmul(pkv[:], wkv_sb[:, kc, :], xtb[:],
                                     start=st, stop=sp)
                # v first (ACT+PE are idle here, DVE is the rope backlog):
                # vT copy gates the transposes which feed PV much later
                nc.scalar.copy(vT[0:HD, s0:s0 + 512], pkv[64:128, :])
                for j in range(4):
                    sc = 4 * sq + j
                    pvt = pvt_pool.tile([128, HD + 1], F32, name=f"pvt{sc}",
                                        tag="pvt")
                    nc.tensor.transpose(pvt[:], vT[:, 128 * sc:128 * (sc + 1)],
                                        identity[0:HD + 1, 0:HD + 1])
                    nc.vector.tensor_mul(v128[sq][:, j, :], pvt[:],
                                         ones_sb[:, 0:HD + 1])
                # rope q: t1 = pq*cos to SBUF, then pq *= sin in place (PSUM);
                # combines mix one SBUF + one PSUM input (different base
                # partitions are only legal for mixed-space inputs)
                for m in range(NPAIR):
                    t1 = tmp_pool.tile([128, 512], F32, name=f"t1q{sq}{m}",
                                       tag="t1")
                    nc.vector.tensor_mul(t1[:], pq[m][:], cs_sb[:, s0:s0 + 512])
                    nc.vector.tensor_mul(pq[m][:], pq[m][:],
                                         sc_sb[:, s0:s0 + 512])
                    qd = qTp[m][sq]
                    nc.vector.tensor_sub(qd[0:32, :], t1[0:32, :],
                                         pq[m][64:96, :])
                    nc.vector.tensor_add(qd[32:64, :], pq[m][0:32, :],
                                         t1[64:96, :])
                    nc.vector.tensor_sub(qd[64:96, :], t1[32:64, :],
                                         pq[m][96:128, :])
                    nc.vector.tensor_add(qd[96:128, :], pq[m][32:64, :],
                                         t1[96:128, :])
                # rope k: t1k = [a*cos; b*cos], then pkv[0:64] *= [s;s]
                t1k = tmp_pool.tile([64, 512], F32, name=f"t1k{sq}", tag="t1k")
                nc.vector.tensor_mul(t1k[:], pkv[0:64, :],
                                     cs_sb[0:64, s0:s0 + 512])
                nc.vector.tensor_mul(pkv[0:64, :], pkv[0:64, :],
                                     sc_sb[0:64, s0:s0 + 512])
                kd = kT2[sq]
                nc.vector.tensor_sub(kd[0:32, :], t1k[0:32, :], pkv[32:64, :])
                nc.vector.tensor_add(kd[32:64, :], pkv[0:32, :], t1k[32:64, :])
                # duplicate kv head into partitions 64-127 (row-group packing)
                nc.scalar.copy(kd[64:128, :], kd[0:64, :])

        # ---------------- Stage A (attention) + interleaved Stage W --------
        with ExitStack() as ctx:
            psc_pool = ctx.enter_context(
                tc.tile_pool(name="psc", bufs=2, space="PSUM"))
            po_pool = ctx.enter_context(
                tc.tile_pool(name="po", bufs=1, space="PSUM"))
            pw_pool = ctx.enter_context(
                tc.tile_pool(name="pw", bufs=2, space="PSUM"))
            probs_pool = ctx.enter_context(tc.tile_pool(name="probs", bufs=6))
            nrm_pool = ctx.enter_context(tc.tile_pool(name="nrm", bufs=4))
            osb_pool = ctx.enter_context(tc.tile_pool(name="osb", bufs=4))

            def w_block(sb, tail=False):
                """Stage W s-block: out[sb] = sum_p attnT[p][t].T @ wo_sb[:,p]
                with t = sb//4."""
                t, c0 = sb // 4, 128 * (sb % 4)
                for n in range(4):
                    pw = pw_pool.tile([128, 512], F32, name=f"pw{sb}_{n}",
                                      tag="pw")
                    for p in range(NPAIR):
                        nc.tensor.matmul(pw[:], attnT[p][t][:, c0:c0 + 128],
                                         wo_sb[:, p, 512 * n:512 * (n + 1)],
                                         start=(p == 0), stop=(p == 1))
                    osb = osb_pool.tile([128, 512], F16, name=f"osb{sb}_{n}",
                                        tag="osb")
                    # cheap convert-copy: TENSOR_TENSOR mul-by-ones on DVE is
                    # ~2x faster than TENSOR_SCALAR; ACT is the exp bottleneck
                    # during attention so it only helps on the tail
                    if tail and n % 2 == 1:
                        nc.scalar.copy(osb[:], pw[:])
                    else:
                        nc.vector.tensor_mul(osb[:], pw[:], ones_sb[:])
                    nc.sync.dma_start(out_d.ap()[sb, :, 512 * n:512 * (n + 1)],
                                      osb[:])

            def attn_tile(p, t, w_blocks=()):
                """Process q-tile t for head pair p; software-pipelined:
                scores run one block ahead of PV. w_blocks are stage-W
                s-blocks to intersperse (emitted between attention blocks)."""
                nb = 4 * t + 4
                po = [po_pool.tile([128, 512], F32, name=f"po{p}{t}{h}",
                                   tag=f"po{h}") for h in range(2)]
                wb = list(w_blocks)
                state = []  # (b, col0, probs)

                def emit_scores(b):
                    j = max(0, b - 4 * t)
                    col0 = 128 * j
                    diag = b >= 4 * t
                    psc = psc_pool.tile([128, 2, 512], F32,
                                        name=f"psc{p}{t}{b}", tag="psc")
                    for h in range(2):
                        nc.tensor.matmul(
                            psc[:, h, col0:512],
                            kT2[b // 4][64 * h:64 * (h + 1),
                                        128 * (b % 4):128 * (b % 4 + 1)],
                            qTp[p][t][64 * h:64 * (h + 1), col0:512],
                            start=True, stop=not diag)
                        if diag:
                            # causal mask folded into the accumulation:
                            # += I.T @ maskM adds -1e9 where key > query,
                            # so exp gives exact zeros (no DVE op needed)
                            nc.tensor.matmul(
                                psc[:, h, col0:col0 + 128], idT_sb[:],
                                maskM_sb[:], start=False, stop=True)
                    probs = probs_pool.tile([128, 2, 512], BF16,
                                            name=f"pr{p}{t}{b}", tag="probs")
                    nc.scalar.activation(probs[:, :, col0:512],
                                         psc[:, :, col0:512], AF.Exp,
                                         scale=0.125)
                    state.append((b, col0, probs))

                def emit_pv(b, col0, probs):
                    for h in range(2):
                        nc.tensor.matmul(po[h][0:HD + 1, col0:512],
                                         v128[b // 4][:, b % 4, :],
                                         probs[:, h, col0:512],
                                         start=(b == 0), stop=(b == nb - 1))

                emit_scores(0)
                for b in range(1, nb):
                    emit_scores(b)
                    emit_pv(*state.pop(0))
                    if wb and b % 3 == 0:
                        w_block(wb.pop(0))
                emit_pv(*state.pop(0))
                for sb in wb:
                    w_block(sb)
                # normalize: po row 64 holds the softmax denominators
                for h in range(2):
                    den = nrm_pool.tile([1, 512], F32, name=f"dn{p}{t}{h}",
                                        tag="den")
                    nc.scalar.copy(den[:], po[h][HD:HD + 1, :])
                    recip = nrm_pool.tile([1, 512], F32, name=f"rc{p}{t}{h}",
                                          tag="recip")
                    nc.vector.reciprocal_approx_fast(recip[:], den[:])
                    rfac = nrm_pool.tile([HD, 512], F32, name=f"rf{p}{t}{h}",
                                         tag="rfac")
                    nc.gpsimd.partition_broadcast(rfac[:], recip[:])
                    nc.vector.tensor_mul(attnT[p][t][64 * h:64 * h + HD, :],
                                         po[h][0:HD, :], rfac[:])

            attn_tile(0, 0)
            attn_tile(1, 0)
            attn_tile(0, 1)
            attn_tile(1, 1)
            # s-blocks 0-7 need only t<=1 of all heads; 8-11 need t=2 too.
            # intersperse them so stage-W matmuls fill PE gaps under ACT.
            attn_tile(0, 2, w_blocks=(0, 1, 2))
            attn_tile(1, 2, w_blocks=(3, 4, 5))
            attn_tile(0, 3, w_blocks=(6, 7, 8, 9))
            attn_tile(1, 3, w_blocks=(10, 11))
            for sb in (12, 13, 14, 15):
                w_block(sb, tail=True)

    nc.compile()
    return nc


_NC_CACHE = None
LAST_RESULT = None


def _get_nc():
    global _NC_CACHE
    if _NC_CACHE is None:
        _NC_CACHE = _build()
    return _NC_CACHE


def _pair_permute(w, h0, h1):
    """wq cols of heads h0,h1 -> [h0_ev(32) | h1_ev(32) | h0_od(32) | h1_od(32)]."""
    c0 = w[:, HD * h0:HD * (h0 + 1)]
    c1 = w[:, HD * h1:HD * (h1 + 1)]
    return np.concatenate(
        [c0[:, 0::2], c1[:, 0::2], c0[:, 1::2], c1[:, 1::2]], axis=1)


def kernel(x, wq, wk, wv, wo, freqs_cos, freqs_sin, mask, start_pos=0):
    assert int(start_pos) == 0, "kernel specialized for start_pos == 0"
    import ml_dtypes
    x = np.asarray(x, np.float32)
    b, s, d = x.shape
    assert (b, s, d) == (1, S, D)
    xT = np.ascontiguousarray(x[0].T).astype(ml_dtypes.bfloat16)
    # pre-tile: xT[sq, kc] = contiguous (128, 512) block -> 1-descriptor DMAs
    xTt = np.ascontiguousarray(
        xT.reshape(16, 128, 4, 512).transpose(2, 0, 1, 3))
    wq = np.asarray(wq, np.float32)
    wk = np.asarray(wk, np.float32)
    wv = np.asarray(wv, np.float32)
    wo = np.asarray(wo, np.float32)
    cT = np.asarray(freqs_cos, np.float32).T    # (32, S)
    sT = np.asarray(freqs_sin, np.float32).T
    csT = np.ascontiguousarray(np.concatenate([cT] * 4, axis=0))  # all-cos
    scT = np.ascontiguousarray(np.concatenate([sT] * 4, axis=0))  # all-sin
    # maskM[k, q] = -1e9 where key k > query q (strictly-lower in [k, q])
    maskM = np.ascontiguousarray(
        np.where(np.asarray(mask, np.float32)[:128, :128].T == 0.0,
                 np.float32(0.0), np.float32(-1e9))
    ).astype(ml_dtypes.bfloat16)
    idT = np.eye(128, dtype=np.float32).astype(ml_dtypes.bfloat16)

    in_maps = []
    for c in range(N_CORES):
        # wq pair-packed: [128, 16, pair, 128]
        wq_c = np.stack([_pair_permute(wq, 4 * c + 2 * m, 4 * c + 2 * m + 1)
                         for m in range(NPAIR)], axis=1)  # (D, 2, 128)
        wq_c = np.ascontiguousarray(
            wq_c.reshape(16, 128, NPAIR, 128).transpose(1, 0, 2, 3)
        ).astype(ml_dtypes.bfloat16)
        # wkv: k permuted [ev|od] cols 0-63, v natural cols 64-127
        wk_c = wk[:, HD * c:HD * (c + 1)]
        wk_p = np.concatenate([wk_c[:, 0::2], wk_c[:, 1::2]], axis=1)
        wkv_c = np.ascontiguousarray(
            np.concatenate([wk_p, wv[:, HD * c:HD * (c + 1)]], axis=1)
            .reshape(16, 128, 128).transpose(1, 0, 2)).astype(ml_dtypes.bfloat16)
        # wo rows for this core's 4 heads, split per pair: [128, pair, D]
        wo_c = np.ascontiguousarray(
            wo[256 * c:256 * (c + 1)].reshape(NPAIR, 128, D).transpose(1, 0, 2)
        ).astype(ml_dtypes.bfloat16)
        in_maps.append({
            "xT": xTt,
            "wq": wq_c,
            "wkv": wkv_c,
            "wo": wo_c,
            "csT": csT,
            "scT": scT,
            "maskM": maskM,
            "idT": idT,
        })

    nc = _get_nc()
    res = bass_utils.run_bass_kernel_spmd(
        nc, in_maps, core_ids=list(range(N_CORES)),
        trace=bool(os.environ.get("BASS_TRACE")))
    global LAST_RESULT
    LAST_RESULT = res
    acc = np.zeros((16, 128, D), np.float32)
    for c in range(N_CORES):
        acc += res.results[c]["out"].astype(np.float32)
    return acc.reshape(1, S, D)


# revision 25
# speedup vs baseline: 1.0271x; 1.0271x over previous
"""GQA attention (S=2048, D=2048, 32 q-heads / 8 kv-heads, rope, causal) on 8
Trainium2 NeuronCores, tensor-parallel over heads (1 kv head + 4 q heads per
core). No collectives: each core computes a full (2048, 2048) partial output
attn_c.T @ wo_c (f16) and the host sums the 8 partials (unsharding a
head-sharded contraction).

Layout notes (activations on-chip live in the transposed/"T" domain):
 - xT (D,S) host-transposed so the contraction dim D is the SBUF partition dim.
 - q weights are column-permuted per head PAIR: [hA_ev|hB_ev|hA_od|hB_od] so
   rope becomes 2 full-width muls + 4 combines per pair psum; scores are
   permutation-invariant per head. k likewise [ev|od].
 - qT pair tiles hold head A in partitions 0-63, head B in 64-127; kT2 holds
   the kv head duplicated in both halves -> the two heads' K=64 score matmuls
   run concurrently in different PE row-groups (tile_position auto-derived
   from base partitions).
 - kT2/qTp/v128/attnT are split per 512-column chunk so tile-granular
   dependencies don't serialize stages: attention q-tile t only waits on
   projection chunk t, and out s-block sb only waits on attention tile
   t = sb//4 of each head.
 - scoresT[s,q] per 128-row s-block; the pair's two score blocks land in one
   [128,2,512] PSUM tile so a single ACTIVATE exps both (halves ACT count).
 - softmax denominators come free from a ones-row appended to v (row 64 of the
   PV psum); no max-subtraction (scores*0.125 ~ N(0,1), exp safe in f32).
 - causal masking: s-blocks strictly above the diagonal are skipped; the
   diagonal 128x128 sub-block gets a 0/1 mask multiply post-exp.
 - stage W s-blocks are interspersed into late attention tiles to fill PE
   gaps while ACT works through the exps.
"""
import os
import sys
from contextlib import ExitStack

import numpy as np

try:
    import concourse.bass as bass  # noqa: F401
except ImportError:  # platform tree not on sys.path in a fresh dir
    sys.path.insert(0, "/opt/trn_rl_repo")
    import concourse.bass as bass  # noqa: F401

import concourse.mybir as mybir
from concourse import bacc, bass_utils, tile
from concourse.masks import make_identity

F32 = mybir.dt.float32
F16 = mybir.dt.float16
BF16 = mybir.dt.bfloat16
AF = mybir.ActivationFunctionType

S = 2048          # sequence length
D = 2048          # model dim
HD = 64           # head dim
N_CORES = 8
NPAIR = 2         # head pairs per core (4 q heads)


def _build():
    nc = bacc.Bacc("TRN2", target_bir_lowering=False, debug=False,
                   num_devices=N_CORES)
    xT_d = nc.dram_tensor("xT", [4, 16, 128, 512], BF16, kind="ExternalInput")
    wq_d = nc.dram_tensor("wq", [128, 16, NPAIR, 128], BF16,
                          kind="ExternalInput")
    wkv_d = nc.dram_tensor("wkv", [128, 16, 128], BF16, kind="ExternalInput")
    wo_d = nc.dram_tensor("wo", [128, NPAIR, D], BF16, kind="ExternalInput")
    cs_d = nc.dram_tensor("csT", [128, S], F32, kind="ExternalInput")
    sc_d = nc.dram_tensor("scT", [128, S], F32, kind="ExternalInput")
    mask_d = nc.dram_tensor("maskM", [128, 128], BF16, kind="ExternalInput")
    id_d = nc.dram_tensor("idT", [128, 128], BF16, kind="ExternalInput")
    out_d = nc.dram_tensor("out", [16, 128, D], F16, kind="ExternalOutput")

    with tile.TileContext(nc) as tc, ExitStack() as top:
        persist = top.enter_context(tc.tile_pool(name="persist", bufs=1))
        # per-sq-chunk tiles (fine-grained cross-stage deps)
        qTp = [[persist.tile([128, 512], BF16, name=f"qTp{m}_{sq}",
                             uniquify=False) for sq in range(4)]
               for m in range(NPAIR)]
        kT2 = [persist.tile([128, 512], BF16, name=f"kT2_{sq}",
                            uniquify=False) for sq in range(4)]
        v128 = [persist.tile([128, 4, HD + 1], BF16, name=f"v128_{sq}",
                             uniquify=False) for sq in range(4)]
        # attnT[pair][t]: [128, 512]; rows 0-63 head 2p, 64-127 head 2p+1
        attnT = [[persist.tile([128, 512], BF16, name=f"attnT{p}{t}",
                               uniquify=False) for t in range(4)]
                 for p in range(NPAIR)]
        maskM_sb = persist.tile([128, 128], BF16, name="maskM_sb")
        idT_sb = persist.tile([128, 128], BF16, name="idT_sb")
        wo_sb = persist.tile([128, NPAIR, D], BF16, name="wo_sb")
        cs_sb = persist.tile([128, S], F32, name="cs_sb")
        sc_sb = persist.tile([128, S], F32, name="sc_sb")
        ones_sb = persist.tile([128, 512], BF16, name="ones_sb")
        nc.vector.memset(ones_sb[:], 1.0)

        # ---------------- Stage P: q/k/v projections + rope ----------------
        with ExitStack() as ctx:
            wpool = ctx.enter_context(tc.tile_pool(name="wpool", bufs=1))
            wq_sb = wpool.tile([128, 16, NPAIR, 128], BF16, name="wq_sb")
            wkv_sb = wpool.tile([128, 16, 128], BF16, name="wkv_sb")
            vT = wpool.tile([HD + 1, S], F32, name="vT")
            identity = wpool.tile([128, 128], F32, name="identity")
            make_identity(nc, identity[:])
            nc.sync.dma_start(wq_sb[:], wq_d.ap())
            nc.scalar.dma_start(wkv_sb[:], wkv_d.ap())

            xtb_pool = ctx.enter_context(tc.tile_pool(name="xtb", bufs=10))
            pq_pool = ctx.enter_context(
                tc.tile_pool(name="pq", bufs=4, space="PSUM"))
            pkv_pool = ctx.enter_context(
                tc.tile_pool(name="pkv", bufs=2, space="PSUM"))
            pvt_pool = ctx.enter_context(
                tc.tile_pool(name="pvt", bufs=2, space="PSUM"))
            tmp_pool = ctx.enter_context(tc.tile_pool(name="ropetmp", bufs=2))

            nc.vector.memset(vT[HD:HD + 1, :], 1.0)

            for sq in range(4):
                s0 = 512 * sq
                pq = [pq_pool.tile([128, 512], F32, name=f"pq{sq}_{m}",
                                   tag="pq") for m in range(NPAIR)]
                pkv = pkv_pool.tile([128, 512], F32, name=f"pkv{sq}",
                                    tag="pkv")
                for kc in range(16):
                    xtb = xtb_pool.tile([128, 512], BF16,
                                        name=f"xtb{sq}_{kc}", tag="xtb")
                    eng = nc.sync if kc % 2 == 0 else nc.scalar
                    eng.dma_start(xtb[:], xT_d.ap()[sq, kc])
                    if sq == 0 and kc == 15:
                        # prefetches AFTER the hot-path DMAs so their
                        # descriptors don't grab the SDMA engines first
                        nc.gpsimd.dma_start(maskM_sb[:], mask_d.ap())
                        nc.gpsimd.dma_start(idT_sb[:], id_d.ap())
                        nc.gpsimd.dma_start(cs_sb[:], cs_d.ap())
                        nc.gpsimd.dma_start(sc_sb[:], sc_d.ap())
                        nc.gpsimd.dma_start(wo_sb[:], wo_d.ap())
                    st, sp = (kc == 0), (kc == 15)
                    for m in range(NPAIR):
                        nc.tensor.matmul(pq[m][:], wq_sb[:, kc, m, :],
                                         xtb[:], start=st, stop=sp)
                    nc.tensor.matmul(pkv[:], wkv_sb[:, kc, :], xtb[:],
                                     start=st, stop=sp)
                # v first (ACT+PE are idle here, DVE is the rope backlog):
                # vT copy gates the transposes which feed PV much later
                nc.scalar.copy(vT[0:HD, s0:s0 + 512], pkv[64:128, :])
                for j in range(4):
                    sc = 4 * sq + j
                    pvt = pvt_pool.tile([128, HD + 1], F32, name=f"pvt{sc}",
                                        tag="pvt")
                    nc.tensor.transpose(pvt[:], vT[:, 128 * sc:128 * (sc + 1)],
                                        identity[0:HD + 1, 0:HD + 1])
                    nc.vector.tensor_mul(v128[sq][:, j, :], pvt[:],
                                         ones_sb[:, 0:HD + 1])
                # rope q: t1 = pq*cos to SBUF, then pq *= sin in place (PSUM);
                # combines mix one SBUF + one PSUM input (different base
                # partitions are only legal for mixed-space inputs)
                for m in range(NPAIR):
                    t1 = tmp_pool.tile([128, 512], F32, name=f"t1q{sq}{m}",
                                       tag="t1")
                    nc.vector.tensor_mul(t1[:], pq[m][:], cs_sb[:, s0:s0 + 512])
                    nc.vector.tensor_mul(pq[m][:], pq[m][:],
                                         sc_sb[:, s0:s0 + 512])
                    qd = qTp[m][sq]
                    nc.vector.tensor_sub(qd[0:32, :], t1[0:32, :],
                                         pq[m][64:96, :])
                    nc.vector.tensor_add(qd[32:64, :], pq[m][0:32, :],
                                         t1[64:96, :])
                    nc.vector.tensor_sub(qd[64:96, :], t1[32:64, :],
                                         pq[m][96:128, :])
                    nc.vector.tensor_add(qd[96:128, :], pq[m][32:64, :],
                                         t1[96:128, :])
                # rope k: t1k = [a*cos; b*cos], then pkv[0:64] *= [s;s]
                t1k = tmp_pool.tile([64, 512], F32, name=f"t1k{sq}", tag="t1k")
                nc.vector.tensor_mul(t1k[:], pkv[0:64, :],
                                     cs_sb[0:64, s0:s0 + 512])
                nc.vector.tensor_mul(pkv[0:64, :], pkv[0:64, :],
                                     sc_sb[0:64, s0:s0 + 512])
                kd = kT2[sq]
                nc.vector.tensor_sub(kd[0:32, :], t1k[0:32, :], pkv[32:64, :])
                nc.vector.tensor_add(kd[32:64, :], pkv[0:32, :], t1k[32:64, :])
                # duplicate kv head into partitions 64-127 (row-group packing)
                nc.scalar.copy(kd[64:128, :], kd[0:64, :])

        # ---------------- Stage A (attention) + interleaved Stage W --------
        with ExitStack() as ctx:
            psc_pool = ctx.enter_context(
                tc.tile_pool(name="psc", bufs=2, space="PSUM"))
            po_pool = ctx.enter_context(
                tc.tile_pool(name="po", bufs=1, space="PSUM"))
            pw_pool = ctx.enter_context(
                tc.tile_pool(name="pw", bufs=2, space="PSUM"))
            probs_pool = ctx.enter_context(tc.tile_pool(name="probs", bufs=6))
            nrm_pool = ctx.enter_context(tc.tile_pool(name="nrm", bufs=4))
            osb_pool = ctx.enter_context(tc.tile_pool(name="osb", bufs=4))

            def w_block(sb, tail=False):
                """Stage W s-block: out[sb] = sum_p attnT[p][t].T @ wo_sb[:,p]
                with t = sb//4."""
                t, c0 = sb // 4, 128 * (sb % 4)
                for n in range(4):
                    pw = pw_pool.tile([128, 512], F32, name=f"pw{sb}_{n}",
                                      tag="pw")
                    for p in range(NPAIR):
                        nc.tensor.matmul(pw[:], attnT[p][t][:, c0:c0 + 128],
                                         wo_sb[:, p, 512 * n:512 * (n + 1)],
                                         start=(p == 0), stop=(p == 1))
                    osb = osb_pool.tile([128, 512], F16, name=f"osb{sb}_{n}",
                                        tag="osb")
                    # cheap convert-copy: TENSOR_TENSOR mul-by-ones on DVE is
                    # ~2x faster than TENSOR_SCALAR; ACT is the exp bottleneck
                    # during attention so it only helps on the tail
                    if tail and n % 2 == 1:
                        nc.scalar.copy(osb[:], pw[:])
                    else:
                        nc.vector.tensor_mul(osb[:], pw[:], ones_sb[:])
                    nc.sync.dma_start(out_d.ap()[sb, :, 512 * n:512 * (n + 1)],
                                      osb[:])

            def attn_tile(p, t, w_blocks=()):
                """Process q-tile t for head pair p; software-pipelined:
                scores run one block ahead of PV. w_blocks are stage-W
                s-blocks to intersperse (emitted between attention blocks)."""
                nb = 4 * t + 4
                po = [po_pool.tile([128, 512], F32, name=f"po{p}{t}{h}",
                                   tag=f"po{h}") for h in range(2)]
                wb = list(w_blocks)
                state = []  # (b, col0, probs)

                def emit_scores(b):
                    j = max(0, b - 4 * t)
                    col0 = 128 * j
                    diag = b >= 4 * t
                    psc = psc_pool.tile([128, 2, 512], F32,
                                        name=f"psc{p}{t}{b}", tag="psc")
                    for h in range(2):
                        nc.tensor.matmul(
                            psc[:, h, col0:512],
                            kT2[b // 4][64 * h:64 * (h + 1),
                                        128 * (b % 4):128 * (b % 4 + 1)],
                            qTp[p][t][64 * h:64 * (h + 1), col0:512],
                            start=True, stop=not diag)
                        if diag:
                            # causal mask folded into the accumulation:
                            # += I.T @ maskM adds -1e9 where key > query,
                            # so exp gives exact zeros (no DVE op needed)
                            nc.tensor.matmul(
                                psc[:, h, col0:col0 + 128], idT_sb[:],
                                maskM_sb[:], start=False, stop=True)
                    probs = probs_pool.tile([128, 2, 512], BF16,
                                            name=f"pr{p}{t}{b}", tag="probs")
                    nc.scalar.activation(probs[:, :, col0:512],
                                         psc[:, :, col0:512], AF.Exp,
                                         scale=0.125)
                    state.append((b, col0, probs))

                def emit_pv(b, col0, probs):
                    for h in range(2):
                        nc.tensor.matmul(po[h][0:HD + 1, col0:512],
                                         v128[b // 4][:, b % 4, :],
                                         probs[:, h, col0:512],
                                         start=(b == 0), stop=(b == nb - 1))

                emit_scores(0)
                for b in range(1, nb):
                    emit_scores(b)
                    emit_pv(*state.pop(0))
                    if wb and b % 3 == 0:
                        w_block(wb.pop(0))
                emit_pv(*state.pop(0))
                for sb in wb:
                    w_block(sb)
                # normalize: po row 64 holds the softmax denominators
                for h in range(2):
                    den = nrm_pool.tile([1, 512], F32, name=f"dn{p}{t}{h}",
                                        tag="den")
                    nc.scalar.copy(den[:], po[h][HD:HD + 1, :])
                    recip = nrm_pool.tile([1, 512], F32, name=f"rc{p}{t}{h}",
                                          tag="recip")
                    nc.vector.reciprocal_approx_fast(recip[:], den[:])
                    rfac = nrm_pool.tile([HD, 512], F32, name=f"rf{p}{t}{h}",
                                         tag="rfac")
                    nc.gpsimd.partition_broadcast(rfac[:], recip[:])
                    nc.vector.tensor_mul(attnT[p][t][64 * h:64 * h + HD, :],
                                         po[h][0:HD, :], rfac[:])

            # big tiles first: dense PE work right after projections, and
            # their attnT output unlocks stage-W s-blocks (sb//4 = t) that
            # then fill the PE during the small serial-chain-bound tiles
            attn_tile(0, 2)
            attn_tile(1, 2)
            attn_tile(0, 3, w_blocks=(8, 9))
            attn_tile(1, 3, w_blocks=(10, 11))
            attn_tile(0, 0, w_blocks=(12, 13))
            attn_tile(1, 0, w_blocks=(14, 15))
            attn_tile(0, 1, w_blocks=(0, 1))
            attn_tile(1, 1, w_blocks=(2, 3))
            for sb in (4, 5, 6, 7):
                w_block(sb, tail=True)

    nc.compile()
    return nc


_NC_CACHE = None
LAST_RESULT = None


def _get_nc():
    global _NC_CACHE
    if _NC_CACHE is None:
        _NC_CACHE = _build()
    return _NC_CACHE


def _pair_permute(w, h0, h1):
    """wq cols of heads h0,h1 -> [h0_ev(32) | h1_ev(32) | h0_od(32) | h1_od(32)]."""
    c0 = w[:, HD * h0:HD * (h0 + 1)]
    c1 = w[:, HD * h1:HD * (h1 + 1)]
    return np.concatenate(
        [c0[:, 0::2], c1[:, 0::2], c0[:, 1::2], c1[:, 1::2]], axis=1)


def kernel(x, wq, wk, wv, wo, freqs_cos, freqs_sin, mask, start_pos=0):
    assert int(start_pos) == 0, "kernel specialized for start_pos == 0"
    import ml_dtypes
    x = np.asarray(x, np.float32)
    b, s, d = x.shape
    assert (b, s, d) == (1, S, D)
    xT = np.ascontiguousarray(x[0].T).astype(ml_dtypes.bfloat16)
    # pre-tile: xT[sq, kc] = contiguous (128, 512) block -> 1-descriptor DMAs
    xTt = np.ascontiguousarray(
        xT.reshape(16, 128, 4, 512).transpose(2, 0, 1, 3))
    wq = np.asarray(wq, np.float32)
    wk = np.asarray(wk, np.float32)
    wv = np.asarray(wv, np.float32)
    wo = np.asarray(wo, np.float32)
    cT = np.asarray(freqs_cos, np.float32).T    # (32, S)
    sT = np.asarray(freqs_sin, np.float32).T
    csT = np.ascontiguousarray(np.concatenate([cT] * 4, axis=0))  # all-cos
    scT = np.ascontiguousarray(np.concatenate([sT] * 4, axis=0))  # all-sin
    # maskM[k, q] = -1e9 where key k > query q (strictly-lower in [k, q])
    maskM = np.ascontiguousarray(
        np.where(np.asarray(mask, np.float32)[:128, :128].T == 0.0,
                 np.float32(0.0), np.float32(-1e9))
    ).astype(ml_dtypes.bfloat16)
    idT = np.eye(128, dtype=np.float32).astype(ml_dtypes.bfloat16)

    in_maps = []
    for c in range(N_CORES):
        # wq pair-packed: [128, 16, pair, 128]
        wq_c = np.stack([_pair_permute(wq, 4 * c + 2 * m, 4 * c + 2 * m + 1)
                         for m in range(NPAIR)], axis=1)  # (D, 2, 128)
        wq_c = np.ascontiguousarray(
            wq_c.reshape(16, 128, NPAIR, 128).transpose(1, 0, 2, 3)
        ).astype(ml_dtypes.bfloat16)
        # wkv: k permuted [ev|od] cols 0-63, v natural cols 64-127
        wk_c = wk[:, HD * c:HD * (c + 1)]
        wk_p = np.concatenate([wk_c[:, 0::2], wk_c[:, 1::2]], axis=1)
        wkv_c = np.ascontiguousarray(
            np.concatenate([wk_p, wv[:, HD * c:HD * (c + 1)]], axis=1)
            .reshape(16, 128, 128).transpose(1, 0, 2)).astype(ml_dtypes.bfloat16)
        # wo rows for this core's 4 heads, split per pair: [128, pair, D]
        wo_c = np.ascontiguousarray(
            wo[256 * c:256 * (c + 1)].reshape(NPAIR, 128, D).transpose(1, 0, 2)
        ).astype(ml_dtypes.bfloat16)
        in_maps.append({
            "xT": xTt,
            "wq": wq_c,
            "wkv": wkv_c,
            "wo": wo_c,
            "csT": csT,
            "scT": scT,
            "maskM": maskM,
            "idT": idT,
        })

    nc = _get_nc()
    res = bass_utils.run_bass_kernel_spmd(
        nc, in_maps, core_ids=list(range(N_CORES)),
        trace=bool(os.environ.get("BASS_TRACE")))
    global LAST_RESULT
    LAST_RESULT = res
    acc = np.zeros((16, 128, D), np.float32)
    for c in range(N_CORES):
        acc += res.results[c]["out"].astype(np.float32)
    return acc.reshape(1, S, D)
